# revision 2
# baseline (speedup 1.0000x reference)
"""MoE top-2 routing kernel for Trainium2, expert-parallel across 8 NeuronCores.

Strategy (per sharding_hint: expert-parallel, one expert per core):
  - Host computes the router (softmax + top-2 + combine weights) in f32
    numpy and builds the token->expert dispatch: tokens routed to expert c
    are gathered, transposed, cast to bf16, and padded to a common
    capacity Cap (multiple of 128).
  - Each core holds its expert's W1/W2 fully RESIDENT in SBUF as bf16
    (16.8 MB = 131 KB/partition), loaded once per pass, interleaved
    just-in-time with the first token chunk's matmuls. Tokens stream
    through in chunks of Tc=512: hT = silu(W1^T x + b1), yT = W2^T hT + b2.
  - The device output is the raw per-expert FFN result yT [H, Cap] f32;
    the host applies the top-2 combine weight during the scatter-add
    (out[ix] += y.T * w), so the device does no router work at all.

Per-chunk device pipeline (all matmuls bf16 into f32 PSUM):
  phase A: for i in 32 i-tiles: ph[i] = sum_k W1[k,i]^T x[k]  (8 matmuls,
           512 tokens streamed per stationary load), silu-drain -> h (bf16)
  phase B: for hb in 8 h-tiles: py[hb] = sum_i W2[i,hb]^T h[i] (32 matmuls
           accumulated in one PSUM bank), +b2 drain -> y (f32) -> DMA out.

PE work per 512-token chunk = (32*8 + 8*32) matmuls x 512 cycles
= 262144 cycles = 109 us; DMA per chunk is only x in (1 MB) + y out (2 MB),
so the kernel is tensor-engine-bound with weights resident.
"""

import numpy as np
import ml_dtypes

import concourse.bacc as bacc
import concourse.tile as tile
import concourse.mybir as mybir
from concourse import bass_utils

BF16NP = ml_dtypes.bfloat16
F32 = mybir.dt.float32
BF16 = mybir.dt.bfloat16
AF = mybir.ActivationFunctionType
ALU = mybir.AluOpType

B, S, H, I, E = 4, 2048, 1024, 4096, 8
T = B * S
TOP_K = 2
NCORES = 8
TC = 512            # token chunk (one f32 PSUM bank of free dim)
KH = H // 128       # 8  k-tiles over H (contraction of matmul 1)
NI = I // 128       # 32 i-tiles over I
NH = H // 128       # 8  output h-tiles


def _chunks(cap):
    out, t0 = [], 0
    while t0 < cap:
        tw = min(TC, cap - t0)
        out.append((t0, tw))
        t0 += tw
    return out


def _build_nc(cap, loop_n=None, preload=False):
    """Build the per-core FFN kernel.

    preload=False: weight DMA is emitted just-in-time inside the body
    (matches the single-pass execution the harness grades).
    preload=True: weight DMA is emitted before the For_i loop, so a
    loop-differenced measurement gives the steady-state token time.
    """
    nc = bacc.Bacc(
        "TRN2",
        target_bir_lowering=False,
        debug=False,
        enable_asserts=False,
        num_devices=NCORES,
    )
    xg = nc.dram_tensor("xg", [KH, 128, cap], BF16, kind="ExternalInput").ap()
    w1 = nc.dram_tensor("w1", [NI, 128, KH * 128], BF16, kind="ExternalInput").ap()
    w2 = nc.dram_tensor("w2", [NI, 128, H], BF16, kind="ExternalInput").ap()
    b1r = nc.dram_tensor("b1r", [128, NI], F32, kind="ExternalInput").ap()
    b2r = nc.dram_tensor("b2r", [128, NH], F32, kind="ExternalInput").ap()
    yt = nc.dram_tensor("yt", [NH, 128, cap], F32, kind="ExternalOutput").ap()

    with tile.TileContext(nc) as tc:
        with (
            tc.tile_pool(name="consts", bufs=1) as cpool,
            tc.tile_pool(name="xf", bufs=2) as xf_pool,
            tc.tile_pool(name="hp", bufs=1) as h_pool,
            tc.tile_pool(name="yp", bufs=2) as y_pool,
            tc.tile_pool(name="php", bufs=2, space="PSUM") as ph_pool,
            tc.tile_pool(name="pyp", bufs=2, space="PSUM") as py_pool,
        ):
            consts = cpool.tile([128, NI + NH], F32)
            b1_sb = consts[:, 0:NI]
            b2_sb = consts[:, NI:NI + NH]
            nc.sync.dma_start(b1_sb, b1r[:, :])
            nc.sync.dma_start(b2_sb, b2r[:, :])
            # resident weights: one big tile each, loaded per-i-tile
            w1_sb = cpool.tile([128, NI * KH * 128], BF16)
            w2_sb = cpool.tile([128, NI * H], BF16)

            if preload:
                for i in range(NI):
                    nc.sync.dma_start(
                        w1_sb[:, i * (KH * 128):(i + 1) * (KH * 128)], w1[i]
                    )
                    nc.sync.dma_start(w2_sb[:, i * H:(i + 1) * H], w2[i])

            import contextlib
            loop_cm = (
                tc.For_i(0, loop_n, 1, hint_engines=(mybir.EngineType.PE,))
                if loop_n else contextlib.nullcontext()
            )
            with loop_cm:
                env = dict(
                    xg=xg, w1=w1, w2=w2, yt=yt,
                    b1_sb=b1_sb, b2_sb=b2_sb, w1_sb=w1_sb, w2_sb=w2_sb,
                    xf_pool=xf_pool, h_pool=h_pool, y_pool=y_pool,
                    ph_pool=ph_pool, py_pool=py_pool,
                )
                _emit_body(nc, tc, cap, not preload, env)

    nc.compile()
    return nc


def _emit_body(nc, tc, cap, load_weights, env):
    xg, w1, w2, yt = env["xg"], env["w1"], env["w2"], env["yt"]
    b1_sb, b2_sb = env["b1_sb"], env["b2_sb"]
    w1_sb, w2_sb = env["w1_sb"], env["w2_sb"]
    xf_pool, h_pool, y_pool = env["xf_pool"], env["h_pool"], env["y_pool"]
    ph_pool, py_pool = env["ph_pool"], env["py_pool"]

    for ci, (t0, tw) in enumerate(_chunks(cap)):
        # ---- load x chunk (transposed: H on partitions, bf16) ----
        xf = xf_pool.tile([128, KH * TC], BF16, tag="xf")
        for k in range(KH):
            nc.sync.dma_start(
                xf[:, k * TC:k * TC + tw], xg[k][:, t0:t0 + tw]
            )

        # ---- phase A: hT[i-tile] = silu(W1^T x + b1) ----
        h = h_pool.tile([128, NI * TC], BF16, tag="h")
        for i in range(NI):
            if load_weights and ci == 0:
                # just-in-time resident load: w1[i] right before use, w2[i]
                # behind it on the same queue (needed only in phase B).
                nc.sync.dma_start(
                    w1_sb[:, i * (KH * 128):(i + 1) * (KH * 128)], w1[i]
                )
                nc.sync.dma_start(w2_sb[:, i * H:(i + 1) * H], w2[i])
            ph = ph_pool.tile([128, TC], F32, tag="ph")
            for k in range(KH):
                nc.tensor.matmul(
                    ph[:, :tw],
                    w1_sb[:, (i * KH + k) * 128:(i * KH + k + 1) * 128],
                    xf[:, k * TC:k * TC + tw],
                    start=(k == 0),
                    stop=(k == KH - 1),
                )
            nc.scalar.activation(
                h[:, i * TC:i * TC + tw], ph[:, :tw], AF.Silu,
                bias=b1_sb[:, i:i + 1],
            )

        # ---- phase B: yT[hb] = W2^T hT + b2, accumulated over all 32 i ----
        for hb in range(NH):
            py = py_pool.tile([128, TC], F32, tag="py")
            for i in range(NI):
                nc.tensor.matmul(
                    py[:, :tw],
                    w2_sb[:, i * H + hb * 128:i * H + (hb + 1) * 128],
                    h[:, i * TC:i * TC + tw],
                    start=(i == 0),
                    stop=(i == NI - 1),
                )
            ys = y_pool.tile([128, TC], F32, tag="y")
            nc.scalar.activation(
                ys[:, :tw], py[:, :tw], AF.Identity, bias=b2_sb[:, hb:hb + 1]
            )
            nc.sync.dma_start(yt[hb][:, t0:t0 + tw], ys[:, :tw])


def _route_host(xf, Wr):
    """f32 router identical to the reference: softmax, top-2, renormalize."""
    logits = xf @ Wr
    m = logits.max(-1, keepdims=True)
    e = np.exp(logits - m)
    probs = e / e.sum(-1, keepdims=True)
    sel = np.argsort(-probs, axis=-1, kind="stable")[:, :TOP_K]
    rw = np.take_along_axis(probs, sel, axis=-1)
    rw = rw / rw.sum(-1, keepdims=True)
    return sel, rw


def kernel_ex(x, Wr, W1, b1, W2, b2, trace=False, loop_n=None, preload=False):
    x = np.ascontiguousarray(np.asarray(x, dtype=np.float32))
    Wr = np.asarray(Wr, dtype=np.float32)
    W1 = np.asarray(W1, dtype=np.float32)
    b1 = np.asarray(b1, dtype=np.float32)
    W2 = np.asarray(W2, dtype=np.float32)
    b2 = np.asarray(b2, dtype=np.float32)

    xf = x.reshape(T, H)
    sel, rw = _route_host(xf, Wr)

    idx = []
    wts = []
    for c in range(E):
        hit = sel == c                               # [T, K]
        ix = np.nonzero(hit.any(-1))[0]
        idx.append(ix)
        wts.append(np.where(hit[ix], rw[ix], 0.0).sum(-1).astype(np.float32))
    cap = max(len(ix) for ix in idx)
    cap = max(128, -(-cap // 128) * 128)

    in_maps = []
    for c in range(E):
        ix = idx[c]
        xgT = np.zeros((H, cap), BF16NP)
        xgT[:, :len(ix)] = xf[ix].astype(BF16NP).T
        # w1 sbuf layout: [i-tile][p, k*128+f] = W1[k*128+p, i*128+f]
        w1r = np.ascontiguousarray(
            W1[c].reshape(KH, 128, NI, 128).transpose(2, 1, 0, 3)
            .reshape(NI, 128, KH * 128).astype(BF16NP)
        )
        in_maps.append({
            "xg": np.ascontiguousarray(xgT.reshape(KH, 128, cap)),
            "w1": w1r,
            "w2": np.ascontiguousarray(W2[c].reshape(NI, 128, H).astype(BF16NP)),
            "b1r": np.ascontiguousarray(b1[c].reshape(NI, 128).T),
            "b2r": np.ascontiguousarray(b2[c].reshape(NH, 128).T),
        })

    nc = _build_nc(cap, loop_n=loop_n, preload=preload)
    res = bass_utils.run_bass_kernel_spmd(
        nc, in_maps, core_ids=list(range(NCORES)), trace=trace
    )

    out = np.zeros((T, H), np.float32)
    for c in range(E):
        ix = idx[c]
        yc = res.results[c]["yt"].reshape(H, cap)
        out[ix] += yc.T[:len(ix)] * wts[c][:, None]
    return out.reshape(B, S, H), res


def kernel(**inputs):
    out, _ = kernel_ex(**inputs)
    return out
